# revision 7
# baseline (speedup 1.0000x reference)
"""MoE top-2 routing kernel for 8 Trainium2 NeuronCores.

Strategy (expert-parallel, host dispatch/combine):
  - Host computes gate logits / top-2 routing / softmax combine weights in
    float64 (cheap: [8192,1024]@[1024,8]).
  - Tokens are gathered per expert, SORTED BY COMBINE WEIGHT (descending)
    and padded to a common capacity C (multiple of NT). Core e processes
    all tokens routed to expert e: y = silu(x @ w1[e]) @ w2[e], bf16 with
    fp32 PSUM accum.
  - fp8 tail tile: the LAST token tile (the C/4 lowest-combine-weight
    tokens of each expert) runs stage 1 in fp8(e4m3) DoubleRow matmuls at
    2x PE throughput. Their small combine weights scale down the fp8
    quantization error; measured end-to-end rel err 1.39e-2 vs the 2e-2
    budget (bf16 everywhere: 3.4e-3). w1 is pre-scaled by 512 on the host
    so its values sit in e4m3's normal range; the silu activation's input
    scale folds the 1/512 back out, so h and stage 2 are unchanged.
  - Device layout avoids all transposes: the kernel computes
    hT = w1.T @ xT and yT = w2.T @ hT, so both weights are consumed in
    their native [K, M] layouts and the host supplies xT (tokens on the
    free axis).
  - Host applies the per-(token, expert) combine weight and scatter-adds
    the two expert outputs per token. yT is stored bf16 (host upcasts).

Hardcoded problem shape: x [4, 2048, 1024], gate_w [1024, 8],
w1 [8, 1024, 4096], w2 [8, 4096, 1024], fp32, TOP_K=2.
"""

import os

import ml_dtypes
import numpy as np

import concourse.bass as bass
from concourse import bacc
import concourse.mybir as mybir
import concourse.tile as tile
from concourse.bass_utils import run_bass_kernel_spmd

BF16 = ml_dtypes.bfloat16
FP8 = ml_dtypes.float8_e4m3

B, S, D, F, E = 4, 2048, 1024, 4096, 8
T = B * S
TOP_K = 2
N_CORES = 8
P = 128          # partitions
NT = 512         # token tile (matmul moving free dim)
D_TILES = D // P    # 8
F_TILES = F // P    # 32
W1_CHUNK = 512      # w1 SBUF tile free size (f), for early compute start
SW1 = 512.0         # fp8 weight pre-scale (power of 2)

# Results of the last kernel() call (timing etc), for test harness use.
LAST = {}


def _routing(x, gate_w):
    """Top-2 routing in float64. Returns (top2 idx [T,2], probs [T,2])."""
    xt = x.reshape(T, D).astype(np.float64)
    logits = xt @ gate_w.astype(np.float64)
    top2 = np.argpartition(-logits, 2, axis=1)[:, :2]
    l2 = np.take_along_axis(logits, top2, 1)
    swap = l2[:, 0] < l2[:, 1]
    top2[swap] = top2[swap][:, ::-1]
    l2 = np.take_along_axis(logits, top2, 1)
    w = np.exp(l2 - l2.max(1, keepdims=True))
    w /= w.sum(1, keepdims=True)
    return top2.astype(np.int32), w.astype(np.float32)


def _build_module(C, fp8_tail, silu_mode="silu"):
    """Build the SPMD Bass module: one expert MLP over C tokens.

    fp8_tail: run stage 1 of the last NT-token tile in fp8 DoubleRow.
    silu_mode: "silu" uses the ACT Silu LUT; "sigmoid_mul" composes
    sigmoid (ACT) and multiply (DVE) — used for CoreSim validation, which
    lacks a Silu implementation.
    """
    nc = bacc.Bacc("TRN2", target_bir_lowering=False, debug=False,
                   enable_asserts=False, num_devices=N_CORES)

    xT = nc.dram_tensor("xT", [D, C], mybir.dt.bfloat16, kind="ExternalInput").ap()
    w1 = nc.dram_tensor("w1", [D, F], mybir.dt.bfloat16, kind="ExternalInput").ap()
    w2 = nc.dram_tensor("w2", [F, D], mybir.dt.bfloat16, kind="ExternalInput").ap()
    if fp8_tail:
        w1q8 = nc.dram_tensor("w1q8", [D, F], mybir.dt.float8e4,
                              kind="ExternalInput").ap()
        x8T = nc.dram_tensor("x8T", [D, NT], mybir.dt.float8e4,
                             kind="ExternalInput").ap()
    yT = nc.dram_tensor("yT", [D, C], mybir.dt.bfloat16, kind="ExternalOutput").ap()

    # token tiles: full NT tiles plus one remainder tile
    tok_tiles = [(i * NT, NT) for i in range(C // NT)]
    if C % NT:
        tok_tiles.append((C - C % NT, C % NT))
    fp8_it = len(tok_tiles) - 1 if fp8_tail else -1

    DR = mybir.MatmulPerfMode.DoubleRow

    with tile.TileContext(nc) as tc:
        with (
            tc.tile_pool(name="wpool", bufs=1) as wpool,
            tc.tile_pool(name="xpool", bufs=2) as xpool,
            tc.tile_pool(name="x8pool", bufs=1) as x8pool,
            tc.tile_pool(name="hpool", bufs=1) as hpool,
            tc.tile_pool(name="opool", bufs=3) as opool,
            tc.tile_pool(name="ps1", bufs=4, space="PSUM") as psum1,
            tc.tile_pool(name="ps2", bufs=1, space="PSUM") as psum2,
        ):
            # ---- weight loads (resident for the whole kernel) ----
            # w1 is stored as 8x8 tiles [128, 512], issued chunk-major on
            # the SP HWDGE ring so the DMA completion order matches the
            # stage-1 consumption order (ft ascending): the first matmul
            # only waits for ~1MB. The first token tile's x load leads the
            # SP ring while w1's two narrow head chunks ride the ACT ring
            # (16 small issues — few enough not to backpressure ACT's
            # instruction stream), so the two gates of the very first
            # matmuls fill in parallel.
            x_t0 = xpool.tile([P, D_TILES, NT], mybir.dt.bfloat16, tag="x")
            x_tiles = {0: x_t0}
            for dt in range(D_TILES):
                nc.sync.dma_start(
                    out=x_t0[:, dt, :tok_tiles[0][1]],
                    in_=xT[dt * P:(dt + 1) * P, :tok_tiles[0][1]])

            # w1 chunks: the two narrow head chunks (c<2) ride the ACT
            # ring as 2 wide 3D descriptors each (the ~0.7us/descriptor
            # issue rate, not bandwidth, is what delays the first ft
            # groups); c2/c3/c4/c6/c8 stream chunk-major on the SP HWDGE
            # ring behind x; c5/c7 (whose deadlines are ~37/51us) ride the
            # slow Pool SWDGE ring, taking 2MB off the SP ring's
            # startup-critical stream. (Moving early chunks to Pool
            # measurably starves stage 1 — SWDGE is too slow for those.)
            chunk_widths = [256, 256, 512, 512, 512, 512, 512, 512, 512]
            chunk_off = np.cumsum([0] + chunk_widths).tolist()
            w1_sb = {}    # (dt, ft) -> stationary AP [P, P]
            w1_tiles = {}  # (dt, c) -> tile, for fp8 tag reuse
            for c in (0, 1):
                cw, co = chunk_widths[c], chunk_off[c]
                t = wpool.tile([P, D_TILES, cw], mybir.dt.bfloat16,
                               tag=f"w1h{c}")
                for half in range(2):
                    dts = slice(half * 4, half * 4 + 4)
                    nc.scalar.dma_start(
                        out=t[:, dts, :],
                        in_=w1[half * 4 * P:(half * 4 + 4) * P,
                               co:co + cw].rearrange("(a p) f -> p a f", p=P))
                for k in range(cw // P):
                    for dt in range(D_TILES):
                        w1_sb[dt, (co // P) + k] = t[:, dt, k * P:(k + 1) * P]
            for c in (2, 3, 4, 6, 8, 5, 7):
                cw, co = chunk_widths[c], chunk_off[c]
                eng = nc.gpsimd if c in (5, 7) else nc.sync
                for dt in range(D_TILES):
                    t = wpool.tile([P, cw], mybir.dt.bfloat16,
                                   tag=f"w1_{dt}_{c}")
                    eng.dma_start(out=t,
                                  in_=w1[dt * P:(dt + 1) * P, co:co + cw])
                    w1_tiles[dt, c] = t
                    for k in range(cw // P):
                        w1_sb[dt, (co // P) + k] = t[:, k * P:(k + 1) * P]
            # w2 follows w1 on the same SP ring: loading it concurrently
            # (ACT ring, Pool SWDGE, interleaved) steals HBM bandwidth
            # from the stage-1-critical w1 stream and measurably starves
            # the first matmuls. The late arrival of w2's last tiles
            # (~75-84us) is absorbed by stage 2's ft-outer loop order
            # below, which only needs w2[ft] at ~66 + 0.86*ft us — always
            # after the tile has landed.
            w2_sb = {}
            for ft in range(F_TILES):
                t = wpool.tile([P, D], mybir.dt.bfloat16, tag=f"w2_{ft}")
                nc.sync.dma_start(out=t, in_=w2[ft * P:(ft + 1) * P, :])
                w2_sb[ft] = t

            # ---- fp8 tail-tile loads (Pool SWDGE, after the w1 stream) ----
            # w1q8 SBUF layout: 32 tiles [128, 2, 512] fp8; tile (kp, fc)
            # holds w1q8 rows (2kp+i)*128..+128, f cols fc*512..+512, i.e.
            # the DoubleRow-paired stationary for d-pair kp. All 32 reuse
            # w1 bf16 tags c in {2..5} via pool-tag rotation — each DMA
            # waits (WAR) until stage 1 of the last bf16 tile stops
            # reading that slot (~100us before the fp8 stage 1 needs it),
            # so none of this traffic competes with the startup stream.
            # x8 follows on the same in-order ring, i.e. it is implicitly
            # delayed too.
            if fp8_tail:
                w1q8_sb = {}
                reuse_tags = [(dt, c) for c in (2, 3, 4, 5)
                              for dt in range(D_TILES)]
                for idx in range(32):
                    kp, fc = idx % 4, idx // 4
                    dt, c = reuse_tags[idx]
                    t = wpool.tile([P, 2, W1_CHUNK], mybir.dt.float8e4,
                                   tag=f"w1_{dt}_{c}")
                    for i in range(2):
                        nc.gpsimd.dma_start(
                            out=t[:, i, :],
                            in_=w1q8[(2 * kp + i) * P:(2 * kp + i + 1) * P,
                                     fc * W1_CHUNK:(fc + 1) * W1_CHUNK])
                    w1q8_sb[kp, fc] = t
                x8_t = x8pool.tile([P, 4, 2, NT], mybir.dt.float8e4, tag="x8")
                for kp in range(4):
                    for i in range(2):
                        nc.gpsimd.dma_start(
                            out=x8_t[:, kp, i, :],
                            in_=x8T[(2 * kp + i) * P:(2 * kp + i + 1) * P, :])

            for it, (off, ntok) in enumerate(tok_tiles):
                # per-d-tile 2D DMAs: 3D DMA descriptors only support a
                # single sync-wait command, which the slot-reuse WAR dep
                # exceeds. The ACT ring carries only these small loads, so
                # the issues never backpressure into ACT's silu work.
                if it == fp8_it:
                    x_t = None
                elif it in x_tiles:
                    x_t = x_tiles.pop(it)
                else:
                    x_t = xpool.tile([P, D_TILES, NT], mybir.dt.bfloat16,
                                     tag="x")
                    for dt in range(D_TILES):
                        nc.scalar.dma_start(
                            out=x_t[:, dt, :ntok],
                            in_=xT[dt * P:(dt + 1) * P, off:off + ntok])

                # stage 1: hT[f, tok] = silu(w1.T @ xT)
                h_tiles = []
                for ft in range(F_TILES):
                    ps = psum1.tile([P, NT], mybir.dt.float32, tag="ps1")
                    if it == fp8_it:
                        # fp8 DoubleRow: contraction 256 per matmul, 2x
                        # column rate. PSUM accumulates x8 @ (w1*512).
                        for kp in range(4):
                            w1q_t = w1q8_sb[kp, ft // 4]
                            fo = (ft % 4) * P
                            nc.tensor.matmul(
                                ps[:, :ntok],
                                w1q_t[:, :, fo:fo + P],
                                x8_t[:, kp, :, :ntok],
                                start=(kp == 0), stop=(kp == 3),
                                perf_mode=DR)
                        scale = 1.0 / SW1
                    else:
                        for dt in range(D_TILES):
                            nc.tensor.matmul(
                                ps[:, :ntok],
                                w1_sb[dt, ft],
                                x_t[:, dt, :ntok],
                                start=(dt == 0), stop=(dt == D_TILES - 1))
                        scale = 1.0
                    h = hpool.tile([P, NT], mybir.dt.bfloat16, tag=f"h{ft}")
                    if silu_mode == "silu":
                        nc.scalar.activation(h[:, :ntok], ps[:, :ntok],
                                             mybir.ActivationFunctionType.Silu,
                                             scale=scale)
                    else:
                        sg = opool.tile([P, NT], mybir.dt.float32, tag="sg")
                        nc.scalar.activation(sg[:, :ntok], ps[:, :ntok],
                                             mybir.ActivationFunctionType.Sigmoid,
                                             scale=scale)
                        nc.vector.tensor_mul(h[:, :ntok], ps[:, :ntok],
                                             sg[:, :ntok])
                    h_tiles.append(h)

                # stage 2: yT[d, tok] = w2.T @ hT. ft is the OUTER loop,
                # accumulating 4 d_tiles in 4 PSUM banks concurrently:
                # each w2[ft] is then needed ~0.86*ft us into the stage
                # instead of all 32 within the first ~7us, so the first
                # token tile's stage 2 never waits on the tail of the w2
                # load.
                last_tile = off + ntok >= C
                for half in range(D_TILES // 4):
                    if last_tile and half == D_TILES // 4 - 1:
                        # final half of the kernel: dt2-inner order staggers
                        # the group endings so only one copy+store trails
                        # the last matmul (w2 is long since resident)
                        for j in range(4):
                            dt2 = half * 4 + j
                            ps2 = psum2.tile([P, NT], mybir.dt.float32,
                                             tag=f"ps2_{j}")
                            for ft in range(F_TILES):
                                nc.tensor.matmul(
                                    ps2[:, :ntok],
                                    w2_sb[ft][:, dt2 * P:(dt2 + 1) * P],
                                    h_tiles[ft][:, :ntok],
                                    start=(ft == 0),
                                    stop=(ft == F_TILES - 1))
                            o = opool.tile([P, NT], mybir.dt.bfloat16,
                                           tag=f"o{j}")
                            nc.vector.tensor_copy(o[:, :ntok],
                                                  ps2[:, :ntok])
                            nc.sync.dma_start(
                                out=yT[dt2 * P:(dt2 + 1) * P,
                                       off:off + ntok],
                                in_=o[:, :ntok])
                        continue
                    ps2_tiles = []
                    for j in range(4):
                        ps2 = psum2.tile([P, NT], mybir.dt.float32,
                                         tag=f"ps2_{j}")
                        ps2_tiles.append(ps2)
                    for ft in range(F_TILES):
                        for j in range(4):
                            dt2 = half * 4 + j
                            nc.tensor.matmul(
                                ps2_tiles[j][:, :ntok],
                                w2_sb[ft][:, dt2 * P:(dt2 + 1) * P],
                                h_tiles[ft][:, :ntok],
                                start=(ft == 0), stop=(ft == F_TILES - 1))
                    for j in range(4):
                        dt2 = half * 4 + j
                        o = opool.tile([P, NT], mybir.dt.bfloat16,
                                       tag=f"o{j}")
                        nc.vector.tensor_copy(o[:, :ntok],
                                              ps2_tiles[j][:, :ntok])
                        nc.sync.dma_start(
                            out=yT[dt2 * P:(dt2 + 1) * P, off:off + ntok],
                            in_=o[:, :ntok])
    nc.compile()
    return nc


def kernel(x, gate_w, w1, w2):
    x = np.asarray(x)
    gate_w = np.asarray(gate_w)
    w1 = np.asarray(w1)
    w2 = np.asarray(w2)

    top2, probs = _routing(x, gate_w)

    # token lists per expert, sorted by combine weight descending so the
    # last token tile holds the lowest-weight tokens (fp8 candidates)
    xt = x.reshape(T, D)
    expert_tok = []   # token indices routed to each expert
    expert_prob = []  # combine weight for those tokens
    for e in range(E):
        hit = (top2 == e)
        sel = np.nonzero(hit.any(1))[0]
        pe_ = (probs * hit)[sel].sum(1)
        order = np.argsort(-pe_, kind="stable")
        expert_tok.append(sel[order])
        expert_prob.append(pe_[order])
    counts = np.array([len(s) for s in expert_tok])
    # Capacity: multiple of NT so every token tile is a full-width matmul.
    # A small overflow above C is computed on the host instead of forcing a
    # narrow (LDWEIGHTS-bound) tail tile or an extra full tile on device.
    # Overflow tokens are the lowest-weight ones (sorted order).
    maxc = int(counts.max())
    C = max(NT, -(-maxc // NT) * NT)
    if C - NT >= maxc - 384:
        C -= NT
    fp8_tail = (C == 2048)  # the measured/validated configuration

    nc = _build_module(C, fp8_tail)

    in_maps = []
    for e in range(E):
        sel = expert_tok[e][:C]
        xe = np.zeros((C, D), dtype=np.float32)
        xe[:len(sel)] = xt[sel]
        im = {
            "xT": np.ascontiguousarray(xe.T.astype(BF16)),
            "w1": w1[e].astype(BF16),
            "w2": np.ascontiguousarray(w2[e]).astype(BF16),
        }
        if fp8_tail:
            im["w1q8"] = (w1[e] * SW1).astype(FP8)
            im["x8T"] = np.ascontiguousarray(xe[C - NT:].T.astype(FP8))
        in_maps.append(im)

    trace = os.environ.get("MOE_TRACE") == "1"
    res = run_bass_kernel_spmd(nc, in_maps, core_ids=list(range(N_CORES)),
                               trace=trace)
    LAST.clear()
    LAST["exec_time_ns"] = res.exec_time_ns
    LAST["mean_exec_time_ns"] = res.mean_exec_time_ns
    LAST["results"] = res

    out = np.zeros((T, D), dtype=np.float32)
    for e in range(E):
        sel = expert_tok[e][:C]
        ye = res.results[e]["yT"][:, :len(sel)].T.astype(np.float32)
        out[sel] += expert_prob[e][:len(sel), None] * ye
        if len(expert_tok[e]) > C:  # host-side overflow (a few tokens)
            sel_o = expert_tok[e][C:]
            h = xt[sel_o] @ w1[e]
            h = h / (1.0 + np.exp(-h))
            yo = h @ w2[e]
            out[sel_o] += expert_prob[e][C:, None] * yo
    return out.reshape(B, S, D)


# revision 8
# speedup vs baseline: 1.0186x; 1.0186x over previous
"""MoE top-2 routing kernel for 8 Trainium2 NeuronCores.

Strategy (expert-parallel, host dispatch/combine):
  - Host computes gate logits / top-2 routing / softmax combine weights in
    float64 (cheap: [8192,1024]@[1024,8]).
  - Tokens are gathered per expert, SORTED BY COMBINE WEIGHT (descending)
    and padded to a common capacity C (multiple of NT). Core e processes
    all tokens routed to expert e: y = silu(x @ w1[e]) @ w2[e], bf16 with
    fp32 PSUM accum.
  - fp8 tail tile: the LAST token tile (the C/4 lowest-combine-weight
    tokens of each expert) runs stage 1 in fp8(e4m3) DoubleRow matmuls at
    2x PE throughput. Their small combine weights scale down the fp8
    quantization error; measured end-to-end rel err 1.39e-2 vs the 2e-2
    budget (bf16 everywhere: 3.4e-3). w1 is pre-scaled by 512 on the host
    so its values sit in e4m3's normal range; the silu activation's input
    scale folds the 1/512 back out, so h and stage 2 are unchanged.
  - Device layout avoids all transposes: the kernel computes
    hT = w1.T @ xT and yT = w2.T @ hT, so both weights are consumed in
    their native [K, M] layouts and the host supplies xT (tokens on the
    free axis).
  - Host applies the per-(token, expert) combine weight and scatter-adds
    the two expert outputs per token. yT is stored bf16 (host upcasts).

Hardcoded problem shape: x [4, 2048, 1024], gate_w [1024, 8],
w1 [8, 1024, 4096], w2 [8, 4096, 1024], fp32, TOP_K=2.
"""

import os

import ml_dtypes
import numpy as np

import concourse.bass as bass
from concourse import bacc
import concourse.mybir as mybir
import concourse.tile as tile
from concourse.bass_utils import run_bass_kernel_spmd

BF16 = ml_dtypes.bfloat16
FP8 = ml_dtypes.float8_e4m3

B, S, D, F, E = 4, 2048, 1024, 4096, 8
T = B * S
TOP_K = 2
N_CORES = 8
P = 128          # partitions
NT = 512         # token tile (matmul moving free dim)
D_TILES = D // P    # 8
F_TILES = F // P    # 32
W1_CHUNK = 512      # w1 SBUF tile free size (f), for early compute start
SW1 = 512.0         # fp8 weight pre-scale (power of 2)

# Results of the last kernel() call (timing etc), for test harness use.
LAST = {}


def _routing(x, gate_w):
    """Top-2 routing in float64. Returns (top2 idx [T,2], probs [T,2])."""
    xt = x.reshape(T, D).astype(np.float64)
    logits = xt @ gate_w.astype(np.float64)
    top2 = np.argpartition(-logits, 2, axis=1)[:, :2]
    l2 = np.take_along_axis(logits, top2, 1)
    swap = l2[:, 0] < l2[:, 1]
    top2[swap] = top2[swap][:, ::-1]
    l2 = np.take_along_axis(logits, top2, 1)
    w = np.exp(l2 - l2.max(1, keepdims=True))
    w /= w.sum(1, keepdims=True)
    return top2.astype(np.int32), w.astype(np.float32)


def _build_module(C, fp8_tail, silu_mode="silu"):
    """Build the SPMD Bass module: one expert MLP over C tokens.

    fp8_tail: run stage 1 of the last NT-token tile in fp8 DoubleRow.
    silu_mode: "silu" uses the ACT Silu LUT; "sigmoid_mul" composes
    sigmoid (ACT) and multiply (DVE) — used for CoreSim validation, which
    lacks a Silu implementation.
    """
    nc = bacc.Bacc("TRN2", target_bir_lowering=False, debug=False,
                   enable_asserts=False, num_devices=N_CORES)

    xT = nc.dram_tensor("xT", [D, C], mybir.dt.bfloat16, kind="ExternalInput").ap()
    w1 = nc.dram_tensor("w1", [D, F], mybir.dt.bfloat16, kind="ExternalInput").ap()
    w2 = nc.dram_tensor("w2", [F, D], mybir.dt.bfloat16, kind="ExternalInput").ap()
    if fp8_tail:
        w1q8 = nc.dram_tensor("w1q8", [D, F], mybir.dt.float8e4,
                              kind="ExternalInput").ap()
        x8T = nc.dram_tensor("x8T", [D, NT], mybir.dt.float8e4,
                             kind="ExternalInput").ap()
    yT = nc.dram_tensor("yT", [D, C], mybir.dt.bfloat16, kind="ExternalOutput").ap()

    # token tiles: full NT tiles plus one remainder tile
    tok_tiles = [(i * NT, NT) for i in range(C // NT)]
    if C % NT:
        tok_tiles.append((C - C % NT, C % NT))
    fp8_it = len(tok_tiles) - 1 if fp8_tail else -1

    DR = mybir.MatmulPerfMode.DoubleRow

    with tile.TileContext(nc) as tc:
        with (
            tc.tile_pool(name="wpool", bufs=1) as wpool,
            tc.tile_pool(name="xpool", bufs=2) as xpool,
            tc.tile_pool(name="x8pool", bufs=1) as x8pool,
            tc.tile_pool(name="hpool", bufs=1) as hpool,
            tc.tile_pool(name="opool", bufs=3) as opool,
            tc.tile_pool(name="ps1", bufs=4, space="PSUM") as psum1,
            tc.tile_pool(name="ps2", bufs=1, space="PSUM") as psum2,
        ):
            # ---- weight loads (resident for the whole kernel) ----
            # w1 is stored as 8x8 tiles [128, 512], issued chunk-major on
            # the SP HWDGE ring so the DMA completion order matches the
            # stage-1 consumption order (ft ascending): the first matmul
            # only waits for ~1MB. The first token tile's x load leads the
            # SP ring while w1's two narrow head chunks ride the ACT ring
            # (16 small issues — few enough not to backpressure ACT's
            # instruction stream), so the two gates of the very first
            # matmuls fill in parallel.
            x_t0 = xpool.tile([P, D_TILES, NT], mybir.dt.bfloat16, tag="x")
            x_tiles = {0: x_t0}
            for dt in range(D_TILES):
                nc.sync.dma_start(
                    out=x_t0[:, dt, :tok_tiles[0][1]],
                    in_=xT[dt * P:(dt + 1) * P, :tok_tiles[0][1]])

            # w1 chunks: head chunks (c<2) ride the ACT ring (few issues,
            # no silu backpressure); the rest stream chunk-major on the SP
            # HWDGE ring behind x, so DMA completion order matches stage-1
            # consumption order. Attempts to parallelize the stream
            # (chunks on Pool SWDGE, wide 3D head descriptors) regressed:
            # any concurrent traffic steals HBM bandwidth from the head of
            # the stream, and wide descriptors delay their completion
            # semaphore to the whole-descriptor boundary.
            chunk_widths = [256, 256, 512, 512, 512, 512, 512, 512, 512]
            chunk_off = np.cumsum([0] + chunk_widths).tolist()
            w1_sb = {}    # (dt, ft) -> stationary AP [P, P]
            w1_tiles = {}  # (dt, c) -> tile, for fp8 tag reuse
            for c, (cw, co) in enumerate(zip(chunk_widths, chunk_off)):
                for dt in range(D_TILES):
                    t = wpool.tile([P, cw], mybir.dt.bfloat16,
                                   tag=f"w1_{dt}_{c}")
                    eng = nc.scalar if c < 2 else nc.sync
                    eng.dma_start(out=t,
                                  in_=w1[dt * P:(dt + 1) * P, co:co + cw])
                    w1_tiles[dt, c] = t
                    for k in range(cw // P):
                        w1_sb[dt, (co // P) + k] = t[:, k * P:(k + 1) * P]
            # w2 follows w1 on the same SP ring: loading it concurrently
            # (ACT ring, Pool SWDGE, interleaved) steals HBM bandwidth
            # from the stage-1-critical w1 stream and measurably starves
            # the first matmuls. The late arrival of w2's last tiles
            # (~75-84us) is absorbed by stage 2's ft-outer loop order
            # below, which only needs w2[ft] at ~66 + 0.86*ft us — always
            # after the tile has landed.
            w2_sb = {}
            for ft in range(F_TILES):
                t = wpool.tile([P, D], mybir.dt.bfloat16, tag=f"w2_{ft}")
                nc.sync.dma_start(out=t, in_=w2[ft * P:(ft + 1) * P, :])
                w2_sb[ft] = t

            # ---- fp8 tail-tile loads (Pool SWDGE, after the w1 stream) ----
            # w1q8 SBUF layout: 32 tiles [128, 2, 512] fp8; tile (kp, fc)
            # holds w1q8 rows (2kp+i)*128..+128, f cols fc*512..+512, i.e.
            # the DoubleRow-paired stationary for d-pair kp. All 32 reuse
            # w1 bf16 tags c in {2..5} via pool-tag rotation — each DMA
            # waits (WAR) until stage 1 of the last bf16 tile stops
            # reading that slot (~100us before the fp8 stage 1 needs it),
            # so none of this traffic competes with the startup stream.
            # x8 follows on the same in-order ring, i.e. it is implicitly
            # delayed too.
            if fp8_tail:
                w1q8_sb = {}
                reuse_tags = [(dt, c) for c in (2, 3, 4, 5)
                              for dt in range(D_TILES)]
                for idx in range(32):
                    kp, fc = idx % 4, idx // 4
                    dt, c = reuse_tags[idx]
                    t = wpool.tile([P, 2, W1_CHUNK], mybir.dt.float8e4,
                                   tag=f"w1_{dt}_{c}")
                    for i in range(2):
                        nc.gpsimd.dma_start(
                            out=t[:, i, :],
                            in_=w1q8[(2 * kp + i) * P:(2 * kp + i + 1) * P,
                                     fc * W1_CHUNK:(fc + 1) * W1_CHUNK])
                    w1q8_sb[kp, fc] = t
                x8_t = x8pool.tile([P, 4, 2, NT], mybir.dt.float8e4, tag="x8")
                for kp in range(4):
                    for i in range(2):
                        nc.gpsimd.dma_start(
                            out=x8_t[:, kp, i, :],
                            in_=x8T[(2 * kp + i) * P:(2 * kp + i + 1) * P, :])

            for it, (off, ntok) in enumerate(tok_tiles):
                # per-d-tile 2D DMAs: 3D DMA descriptors only support a
                # single sync-wait command, which the slot-reuse WAR dep
                # exceeds. The ACT ring carries only these small loads, so
                # the issues never backpressure into ACT's silu work.
                if it == fp8_it:
                    x_t = None
                elif it in x_tiles:
                    x_t = x_tiles.pop(it)
                else:
                    x_t = xpool.tile([P, D_TILES, NT], mybir.dt.bfloat16,
                                     tag="x")
                    for dt in range(D_TILES):
                        nc.scalar.dma_start(
                            out=x_t[:, dt, :ntok],
                            in_=xT[dt * P:(dt + 1) * P, off:off + ntok])

                # stage 1: hT[f, tok] = silu(w1.T @ xT)
                h_tiles = []
                for ft in range(F_TILES):
                    ps = psum1.tile([P, NT], mybir.dt.float32, tag="ps1")
                    if it == fp8_it:
                        # fp8 DoubleRow: contraction 256 per matmul, 2x
                        # column rate. PSUM accumulates x8 @ (w1*512).
                        for kp in range(4):
                            w1q_t = w1q8_sb[kp, ft // 4]
                            fo = (ft % 4) * P
                            nc.tensor.matmul(
                                ps[:, :ntok],
                                w1q_t[:, :, fo:fo + P],
                                x8_t[:, kp, :, :ntok],
                                start=(kp == 0), stop=(kp == 3),
                                perf_mode=DR)
                        scale = 1.0 / SW1
                    else:
                        for dt in range(D_TILES):
                            nc.tensor.matmul(
                                ps[:, :ntok],
                                w1_sb[dt, ft],
                                x_t[:, dt, :ntok],
                                start=(dt == 0), stop=(dt == D_TILES - 1))
                        scale = 1.0
                    h = hpool.tile([P, NT], mybir.dt.bfloat16, tag=f"h{ft}")
                    if silu_mode == "silu":
                        nc.scalar.activation(h[:, :ntok], ps[:, :ntok],
                                             mybir.ActivationFunctionType.Silu,
                                             scale=scale)
                    else:
                        sg = opool.tile([P, NT], mybir.dt.float32, tag="sg")
                        nc.scalar.activation(sg[:, :ntok], ps[:, :ntok],
                                             mybir.ActivationFunctionType.Sigmoid,
                                             scale=scale)
                        nc.vector.tensor_mul(h[:, :ntok], ps[:, :ntok],
                                             sg[:, :ntok])
                    h_tiles.append(h)

                # stage 2: yT[d, tok] = w2.T @ hT. ft is the OUTER loop,
                # accumulating 4 d_tiles in 4 PSUM banks concurrently:
                # each w2[ft] is then needed ~0.86*ft us into the stage
                # instead of all 32 within the first ~7us, so the first
                # token tile's stage 2 never waits on the tail of the w2
                # load.
                last_tile = off + ntok >= C
                for half in range(D_TILES // 4):
                    if last_tile and half == D_TILES // 4 - 1:
                        # final half of the kernel: dt2-inner order staggers
                        # the group endings so only one copy+store trails
                        # the last matmul (w2 is long since resident)
                        for j in range(4):
                            dt2 = half * 4 + j
                            ps2 = psum2.tile([P, NT], mybir.dt.float32,
                                             tag=f"ps2_{j}")
                            for ft in range(F_TILES):
                                nc.tensor.matmul(
                                    ps2[:, :ntok],
                                    w2_sb[ft][:, dt2 * P:(dt2 + 1) * P],
                                    h_tiles[ft][:, :ntok],
                                    start=(ft == 0),
                                    stop=(ft == F_TILES - 1))
                            o = opool.tile([P, NT], mybir.dt.bfloat16,
                                           tag=f"o{j}")
                            nc.vector.tensor_copy(o[:, :ntok],
                                                  ps2[:, :ntok])
                            nc.sync.dma_start(
                                out=yT[dt2 * P:(dt2 + 1) * P,
                                       off:off + ntok],
                                in_=o[:, :ntok])
                        continue
                    ps2_tiles = []
                    for j in range(4):
                        ps2 = psum2.tile([P, NT], mybir.dt.float32,
                                         tag=f"ps2_{j}")
                        ps2_tiles.append(ps2)
                    for ft in range(F_TILES):
                        for j in range(4):
                            dt2 = half * 4 + j
                            nc.tensor.matmul(
                                ps2_tiles[j][:, :ntok],
                                w2_sb[ft][:, dt2 * P:(dt2 + 1) * P],
                                h_tiles[ft][:, :ntok],
                                start=(ft == 0), stop=(ft == F_TILES - 1))
                    for j in range(4):
                        dt2 = half * 4 + j
                        o = opool.tile([P, NT], mybir.dt.bfloat16,
                                       tag=f"o{j}")
                        nc.vector.tensor_copy(o[:, :ntok],
                                              ps2_tiles[j][:, :ntok])
                        nc.sync.dma_start(
                            out=yT[dt2 * P:(dt2 + 1) * P, off:off + ntok],
                            in_=o[:, :ntok])
    nc.compile()
    return nc


def kernel(x, gate_w, w1, w2):
    x = np.asarray(x)
    gate_w = np.asarray(gate_w)
    w1 = np.asarray(w1)
    w2 = np.asarray(w2)

    top2, probs = _routing(x, gate_w)

    # token lists per expert, sorted by combine weight descending so the
    # last token tile holds the lowest-weight tokens (fp8 candidates)
    xt = x.reshape(T, D)
    expert_tok = []   # token indices routed to each expert
    expert_prob = []  # combine weight for those tokens
    for e in range(E):
        hit = (top2 == e)
        sel = np.nonzero(hit.any(1))[0]
        pe_ = (probs * hit)[sel].sum(1)
        order = np.argsort(-pe_, kind="stable")
        expert_tok.append(sel[order])
        expert_prob.append(pe_[order])
    counts = np.array([len(s) for s in expert_tok])
    # Capacity: multiple of NT so every token tile is a full-width matmul.
    # A small overflow above C is computed on the host instead of forcing a
    # narrow (LDWEIGHTS-bound) tail tile or an extra full tile on device.
    # Overflow tokens are the lowest-weight ones (sorted order).
    maxc = int(counts.max())
    C = max(NT, -(-maxc // NT) * NT)
    if C - NT >= maxc - 384:
        C -= NT
    fp8_tail = (C == 2048)  # the measured/validated configuration

    nc = _build_module(C, fp8_tail)

    in_maps = []
    for e in range(E):
        sel = expert_tok[e][:C]
        xe = np.zeros((C, D), dtype=np.float32)
        xe[:len(sel)] = xt[sel]
        im = {
            "xT": np.ascontiguousarray(xe.T.astype(BF16)),
            "w1": w1[e].astype(BF16),
            "w2": np.ascontiguousarray(w2[e]).astype(BF16),
        }
        if fp8_tail:
            im["w1q8"] = (w1[e] * SW1).astype(FP8)
            im["x8T"] = np.ascontiguousarray(xe[C - NT:].T.astype(FP8))
        in_maps.append(im)

    trace = os.environ.get("MOE_TRACE") == "1"
    res = run_bass_kernel_spmd(nc, in_maps, core_ids=list(range(N_CORES)),
                               trace=trace)
    LAST.clear()
    LAST["exec_time_ns"] = res.exec_time_ns
    LAST["mean_exec_time_ns"] = res.mean_exec_time_ns
    LAST["results"] = res

    out = np.zeros((T, D), dtype=np.float32)
    for e in range(E):
        sel = expert_tok[e][:C]
        ye = res.results[e]["yT"][:, :len(sel)].T.astype(np.float32)
        out[sel] += expert_prob[e][:len(sel), None] * ye
        if len(expert_tok[e]) > C:  # host-side overflow (a few tokens)
            sel_o = expert_tok[e][C:]
            h = xt[sel_o] @ w1[e]
            h = h / (1.0 + np.exp(-h))
            yo = h @ w2[e]
            out[sel_o] += expert_prob[e][C:, None] * yo
    return out.reshape(B, S, D)


# revision 15
# speedup vs baseline: 1.0911x; 1.0711x over previous
"""MoE top-2 routing kernel for 8 Trainium2 NeuronCores.

Strategy (expert-parallel, host dispatch/combine):
  - Host computes gate logits / top-2 routing / softmax combine weights in
    float64 (cheap: [8192,1024]@[1024,8]).
  - Tokens are gathered per expert, SORTED BY COMBINE WEIGHT (descending)
    and padded to a common capacity C (multiple of NT). Core e processes
    all tokens routed to expert e: y = silu(x @ w1[e]) @ w2[e], bf16 with
    fp32 PSUM accum.
  - fp8 tail tile: the LAST token tile (the C/4 lowest-combine-weight
    tokens of each expert) runs stage 1 in fp8(e4m3) DoubleRow matmuls at
    2x PE throughput. Their small combine weights scale down the fp8
    quantization error; measured end-to-end rel err 1.39e-2 vs the 2e-2
    budget (bf16 everywhere: 3.4e-3). w1 is pre-scaled by 512 on the host
    so its values sit in e4m3's normal range; the silu activation's input
    scale folds the 1/512 back out, so h and stage 2 are unchanged.
  - Device layout avoids all transposes: the kernel computes
    hT = w1.T @ xT and yT = w2.T @ hT, so both weights are consumed in
    their native [K, M] layouts and the host supplies xT (tokens on the
    free axis).
  - Host applies the per-(token, expert) combine weight and scatter-adds
    the two expert outputs per token. yT is stored bf16 (host upcasts).

Hardcoded problem shape: x [4, 2048, 1024], gate_w [1024, 8],
w1 [8, 1024, 4096], w2 [8, 4096, 1024], fp32, TOP_K=2.
"""

import os

import ml_dtypes
import numpy as np

import concourse.bass as bass
from concourse import bacc
import concourse.mybir as mybir
import concourse.tile as tile
from concourse.bass_utils import run_bass_kernel_spmd

BF16 = ml_dtypes.bfloat16
FP8 = ml_dtypes.float8_e4m3

B, S, D, F, E = 4, 2048, 1024, 4096, 8
T = B * S
TOP_K = 2
N_CORES = 8
P = 128          # partitions
NT = 512         # token tile (matmul moving free dim)
D_TILES = D // P    # 8
F_TILES = F // P    # 32
W1_CHUNK = 512      # w1 SBUF tile free size (f), for early compute start
SW1 = 512.0         # fp8 weight pre-scale (power of 2)

# Results of the last kernel() call (timing etc), for test harness use.
LAST = {}


def _routing(x, gate_w):
    """Top-2 routing in float64. Returns (top2 idx [T,2], probs [T,2])."""
    xt = x.reshape(T, D).astype(np.float64)
    logits = xt @ gate_w.astype(np.float64)
    top2 = np.argpartition(-logits, 2, axis=1)[:, :2]
    l2 = np.take_along_axis(logits, top2, 1)
    swap = l2[:, 0] < l2[:, 1]
    top2[swap] = top2[swap][:, ::-1]
    l2 = np.take_along_axis(logits, top2, 1)
    w = np.exp(l2 - l2.max(1, keepdims=True))
    w /= w.sum(1, keepdims=True)
    return top2.astype(np.int32), w.astype(np.float32)


def _build_module(C, fp8_tail, silu_mode="silu"):
    """Build the SPMD Bass module: one expert MLP over C tokens.

    fp8_tail: run stage 1 of the last NT-token tile in fp8 DoubleRow.
    silu_mode: "silu" uses the ACT Silu LUT; "sigmoid_mul" composes
    sigmoid (ACT) and multiply (DVE) — used for CoreSim validation, which
    lacks a Silu implementation.
    """
    nc = bacc.Bacc("TRN2", target_bir_lowering=False, debug=False,
                   enable_asserts=False, num_devices=N_CORES)

    xT = nc.dram_tensor("xT", [D, C], mybir.dt.bfloat16, kind="ExternalInput").ap()
    w1 = nc.dram_tensor("w1", [D, F], mybir.dt.bfloat16, kind="ExternalInput").ap()
    w2 = nc.dram_tensor("w2", [F, D], mybir.dt.bfloat16, kind="ExternalInput").ap()
    if fp8_tail:
        w1q8 = nc.dram_tensor("w1q8", [D, F], mybir.dt.float8e4,
                              kind="ExternalInput").ap()
        w2q8 = nc.dram_tensor("w2q8", [F, D], mybir.dt.float8e4,
                              kind="ExternalInput").ap()
        x8T = nc.dram_tensor("x8T", [D, NT], mybir.dt.float8e4,
                             kind="ExternalInput").ap()
    yT = nc.dram_tensor("yT", [D, C], mybir.dt.bfloat16, kind="ExternalOutput").ap()

    # token tiles: full NT tiles plus one remainder tile
    tok_tiles = [(i * NT, NT) for i in range(C // NT)]
    if C % NT:
        tok_tiles.append((C - C % NT, C % NT))
    fp8_it = len(tok_tiles) - 1 if fp8_tail else -1

    DR = mybir.MatmulPerfMode.DoubleRow

    with tile.TileContext(nc) as tc:
        with (
            tc.tile_pool(name="wpool", bufs=1) as wpool,
            tc.tile_pool(name="xpool", bufs=2) as xpool,
            tc.tile_pool(name="x8pool", bufs=1) as x8pool,
            tc.tile_pool(name="hpool", bufs=1) as hpool,
            tc.tile_pool(name="opool", bufs=3) as opool,
            tc.tile_pool(name="ps1", bufs=4, space="PSUM") as psum1,
            tc.tile_pool(name="ps2", bufs=1, space="PSUM") as psum2,
        ):
            # ---- weight loads (resident for the whole kernel) ----
            # w1 is stored as 8x8 tiles [128, 512], issued chunk-major on
            # the SP HWDGE ring so the DMA completion order matches the
            # stage-1 consumption order (ft ascending): the first matmul
            # only waits for ~1MB. The first token tile's x load leads the
            # SP ring while w1's two narrow head chunks ride the ACT ring
            # (16 small issues — few enough not to backpressure ACT's
            # instruction stream), so the two gates of the very first
            # matmuls fill in parallel.
            x_t0 = xpool.tile([P, D_TILES, NT], mybir.dt.bfloat16, tag="x")
            x_tiles = {0: x_t0}
            for dt in range(D_TILES):
                nc.sync.dma_start(
                    out=x_t0[:, dt, :tok_tiles[0][1]],
                    in_=xT[dt * P:(dt + 1) * P, :tok_tiles[0][1]])

            # w1 chunks: head chunks (c<2) ride the ACT ring (few issues,
            # no silu backpressure); the rest stream chunk-major on the SP
            # HWDGE ring behind x, so DMA completion order matches stage-1
            # consumption order. Attempts to parallelize the stream
            # (chunks on Pool SWDGE, wide 3D head descriptors) regressed:
            # any concurrent traffic steals HBM bandwidth from the head of
            # the stream, and wide descriptors delay their completion
            # semaphore to the whole-descriptor boundary.
            chunk_widths = [256, 256, 512, 512, 512, 512, 512, 512, 512]
            chunk_off = np.cumsum([0] + chunk_widths).tolist()
            w1_sb = {}    # (dt, ft) -> stationary AP [P, P]
            w1_tiles = {}  # (dt, c) -> tile, for fp8 tag reuse
            for c, (cw, co) in enumerate(zip(chunk_widths, chunk_off)):
                for dt in range(D_TILES):
                    t = wpool.tile([P, cw], mybir.dt.bfloat16,
                                   tag=f"w1_{dt}_{c}")
                    eng = nc.scalar if c < 2 else nc.sync
                    eng.dma_start(out=t,
                                  in_=w1[dt * P:(dt + 1) * P, co:co + cw])
                    w1_tiles[dt, c] = t
                    for k in range(cw // P):
                        w1_sb[dt, (co // P) + k] = t[:, k * P:(k + 1) * P]
            # w2 follows w1 on the same SP ring: loading it concurrently
            # (ACT ring, Pool SWDGE, interleaved) steals HBM bandwidth
            # from the stage-1-critical w1 stream and measurably starves
            # the first matmuls. The late arrival of w2's last tiles
            # (~75-84us) is absorbed by stage 2's ft-outer loop order
            # below, which only needs w2[ft] at ~66 + 0.86*ft us — always
            # after the tile has landed.
            w2_sb = {}
            for ft in range(F_TILES):
                t = wpool.tile([P, D], mybir.dt.bfloat16, tag=f"w2_{ft}")
                nc.sync.dma_start(out=t, in_=w2[ft * P:(ft + 1) * P, :])
                w2_sb[ft] = t

            # ---- fp8 tail-tile loads (Pool SWDGE, after the w1 stream) ----
            # w1q8 SBUF layout: 32 tiles [128, 2, 512] fp8; tile (kp, fc)
            # holds w1q8 rows (2kp+i)*128..+128, f cols fc*512..+512, i.e.
            # the DoubleRow-paired stationary for d-pair kp. All 32 reuse
            # w1 bf16 tags c in {2..5} via pool-tag rotation — each DMA
            # waits (WAR) until stage 1 of the last bf16 tile stops
            # reading that slot (~100us before the fp8 stage 1 needs it),
            # so none of this traffic competes with the startup stream.
            # x8 follows on the same in-order ring, i.e. it is implicitly
            # delayed too.
            if fp8_tail:
                w1q8_sb = {}
                reuse_tags = [(dt, c) for c in (2, 3, 4, 5)
                              for dt in range(D_TILES)]
                for idx in range(32):
                    kp, fc = idx % 4, idx // 4
                    dt, c = reuse_tags[idx]
                    t = wpool.tile([P, 2, W1_CHUNK], mybir.dt.float8e4,
                                   tag=f"w1_{dt}_{c}")
                    for i in range(2):
                        nc.gpsimd.dma_start(
                            out=t[:, i, :],
                            in_=w1q8[(2 * kp + i) * P:(2 * kp + i + 1) * P,
                                     fc * W1_CHUNK:(fc + 1) * W1_CHUNK])
                    w1q8_sb[kp, fc] = t
                x8_t = x8pool.tile([P, 4, 2, NT], mybir.dt.float8e4, tag="x8")
                for kp in range(4):
                    for i in range(2):
                        nc.gpsimd.dma_start(
                            out=x8_t[:, kp, i, :],
                            in_=x8T[(2 * kp + i) * P:(2 * kp + i + 1) * P, :])
                # w2q8: 16 DoubleRow pair tiles [128, 2, 1024] fp8, pair p
                # reusing w2 bf16 tag 2p (same 2KB). Pairs 0-7 follow on
                # the Pool ring here; pairs 8-15 are emitted on the SP
                # ring AFTER the last bf16 tile's stage-2 stores (see the
                # token loop) — a WAR-gated descriptor ahead of store
                # traffic would block the in-order ring for ~250us.
                w2q8_sb = {}
                for p_ in range(16):
                    t = wpool.tile([P, 2, D], mybir.dt.float8e4,
                                   tag=f"w2_{2 * p_}")
                    w2q8_sb[p_] = t
                for p_ in range(8):
                    for i in range(2):
                        nc.gpsimd.dma_start(
                            out=w2q8_sb[p_][:, i, :],
                            in_=w2q8[(2 * p_ + i) * P:(2 * p_ + i + 1) * P, :])

            for it, (off, ntok) in enumerate(tok_tiles):
                # per-d-tile 2D DMAs: 3D DMA descriptors only support a
                # single sync-wait command, which the slot-reuse WAR dep
                # exceeds. The ACT ring carries only these small loads, so
                # the issues never backpressure into ACT's silu work.
                if it == fp8_it:
                    x_t = None
                elif it in x_tiles:
                    x_t = x_tiles.pop(it)
                else:
                    x_t = xpool.tile([P, D_TILES, NT], mybir.dt.bfloat16,
                                     tag="x")
                    for dt in range(D_TILES):
                        nc.scalar.dma_start(
                            out=x_t[:, dt, :ntok],
                            in_=xT[dt * P:(dt + 1) * P, off:off + ntok])

                # stage 1: hT[f, tok] = silu(w1.T @ xT)
                h_tiles = []
                h8_pairs = []
                if it == fp8_it:
                    for p_ in range(16):
                        h8p = hpool.tile([P, 2, NT], mybir.dt.float8e4,
                                         tag=f"h{p_}")
                        h8_pairs.append(h8p)
                for ft in range(F_TILES):
                    ps = psum1.tile([P, NT], mybir.dt.float32, tag="ps1")
                    if it == fp8_it:
                        # fp8 DoubleRow: contraction 256 per matmul, 2x
                        # column rate. PSUM accumulates x8 @ (w1*512).
                        # silu descales and writes fp8 h directly into the
                        # DoubleRow pair slot for the fp8 stage 2.
                        for kp in range(4):
                            w1q_t = w1q8_sb[kp, ft // 4]
                            fo = (ft % 4) * P
                            nc.tensor.matmul(
                                ps[:, :ntok],
                                w1q_t[:, :, fo:fo + P],
                                x8_t[:, kp, :, :ntok],
                                start=(kp == 0), stop=(kp == 3),
                                perf_mode=DR)
                        nc.scalar.activation(
                            h8_pairs[ft // 2][:, ft % 2, :ntok],
                            ps[:, :ntok],
                            mybir.ActivationFunctionType.Silu,
                            scale=1.0 / SW1)
                        continue
                    for dt in range(D_TILES):
                        nc.tensor.matmul(
                            ps[:, :ntok],
                            w1_sb[dt, ft],
                            x_t[:, dt, :ntok],
                            start=(dt == 0), stop=(dt == D_TILES - 1))
                    h = hpool.tile([P, NT], mybir.dt.bfloat16, tag=f"h{ft}")
                    if silu_mode == "silu":
                        nc.scalar.activation(h[:, :ntok], ps[:, :ntok],
                                             mybir.ActivationFunctionType.Silu)
                    else:
                        sg = opool.tile([P, NT], mybir.dt.float32, tag="sg")
                        nc.scalar.activation(sg[:, :ntok], ps[:, :ntok],
                                             mybir.ActivationFunctionType.Sigmoid)
                        nc.vector.tensor_mul(h[:, :ntok], ps[:, :ntok],
                                             sg[:, :ntok])
                    h_tiles.append(h)

                # stage 2: yT[d, tok] = w2.T @ hT. ft is the OUTER loop,
                # accumulating 4 d_tiles in 4 PSUM banks concurrently:
                # each w2[ft] is then needed ~0.86*ft us into the stage
                # instead of all 32 within the first ~7us, so the first
                # token tile's stage 2 never waits on the tail of the w2
                # load.
                last_tile = off + ntok >= C
                if it == fp8_it:
                    nsteps = 16

                    def s2_matmul(ps2, dt2, i):
                        nc.tensor.matmul(
                            ps2[:, :ntok],
                            w2q8_sb[i][:, :, dt2 * P:(dt2 + 1) * P],
                            h8_pairs[i][:, :, :ntok],
                            start=(i == 0), stop=(i == nsteps - 1),
                            perf_mode=DR)
                else:
                    nsteps = F_TILES

                    def s2_matmul(ps2, dt2, i):
                        nc.tensor.matmul(
                            ps2[:, :ntok],
                            w2_sb[i][:, dt2 * P:(dt2 + 1) * P],
                            h_tiles[i][:, :ntok],
                            start=(i == 0), stop=(i == nsteps - 1))

                for half in range(D_TILES // 4):
                    if last_tile and half == D_TILES // 4 - 1:
                        # final half of the kernel: dt2-inner order staggers
                        # the group endings so only one copy+store trails
                        # the last matmul (w2 is long since resident)
                        for j in range(4):
                            dt2 = half * 4 + j
                            ps2 = psum2.tile([P, NT], mybir.dt.float32,
                                             tag=f"ps2_{j}")
                            for i in range(nsteps):
                                s2_matmul(ps2, dt2, i)
                            o = opool.tile([P, NT], mybir.dt.bfloat16,
                                           tag=f"o{j}")
                            nc.vector.tensor_copy(o[:, :ntok],
                                                  ps2[:, :ntok])
                            nc.sync.dma_start(
                                out=yT[dt2 * P:(dt2 + 1) * P,
                                       off:off + ntok],
                                in_=o[:, :ntok])
                        continue
                    ps2_tiles = []
                    for j in range(4):
                        ps2 = psum2.tile([P, NT], mybir.dt.float32,
                                         tag=f"ps2_{j}")
                        ps2_tiles.append(ps2)
                    for i in range(nsteps):
                        for j in range(4):
                            dt2 = half * 4 + j
                            s2_matmul(ps2_tiles[j], dt2, i)
                    for j in range(4):
                        dt2 = half * 4 + j
                        o = opool.tile([P, NT], mybir.dt.bfloat16,
                                       tag=f"o{j}")
                        nc.vector.tensor_copy(o[:, :ntok],
                                              ps2_tiles[j][:, :ntok])
                        nc.sync.dma_start(
                            out=yT[dt2 * P:(dt2 + 1) * P, off:off + ntok],
                            in_=o[:, :ntok])

                # w2q8 pairs 8-15 ride the SP ring, emitted after this
                # (second-to-last) tile's stores so their WAR-gated
                # descriptors never block store traffic on the in-order
                # ring. Their slots free at ~330-343us; stage 2 of the
                # fp8 tile needs them from ~385us.
                if fp8_tail and it == fp8_it - 1:
                    for p_ in range(8, 16):
                        for i in range(2):
                            nc.sync.dma_start(
                                out=w2q8_sb[p_][:, i, :],
                                in_=w2q8[(2 * p_ + i) * P:
                                         (2 * p_ + i + 1) * P, :])
    nc.compile()
    return nc


def kernel(x, gate_w, w1, w2):
    x = np.asarray(x)
    gate_w = np.asarray(gate_w)
    w1 = np.asarray(w1)
    w2 = np.asarray(w2)

    top2, probs = _routing(x, gate_w)

    # token lists per expert, sorted by combine weight descending so the
    # last token tile holds the lowest-weight tokens (fp8 candidates)
    xt = x.reshape(T, D)
    expert_tok = []   # token indices routed to each expert
    expert_prob = []  # combine weight for those tokens
    for e in range(E):
        hit = (top2 == e)
        sel = np.nonzero(hit.any(1))[0]
        pe_ = (probs * hit)[sel].sum(1)
        order = np.argsort(-pe_, kind="stable")
        expert_tok.append(sel[order])
        expert_prob.append(pe_[order])
    counts = np.array([len(s) for s in expert_tok])
    # Capacity: multiple of NT so every token tile is a full-width matmul.
    # A small overflow above C is computed on the host instead of forcing a
    # narrow (LDWEIGHTS-bound) tail tile or an extra full tile on device.
    # Overflow tokens are the lowest-weight ones (sorted order).
    maxc = int(counts.max())
    C = max(NT, -(-maxc // NT) * NT)
    if C - NT >= maxc - 384:
        C -= NT
    fp8_tail = (C == 2048)  # the measured/validated configuration

    nc = _build_module(C, fp8_tail)

    in_maps = []
    for e in range(E):
        sel = expert_tok[e][:C]
        xe = np.zeros((C, D), dtype=np.float32)
        xe[:len(sel)] = xt[sel]
        im = {
            "xT": np.ascontiguousarray(xe.T.astype(BF16)),
            "w1": w1[e].astype(BF16),
            "w2": np.ascontiguousarray(w2[e]).astype(BF16),
        }
        if fp8_tail:
            im["w1q8"] = (w1[e] * SW1).astype(FP8)
            im["w2q8"] = (w2[e] * SW1).astype(FP8)
            im["x8T"] = np.ascontiguousarray(xe[C - NT:].T.astype(FP8))
        in_maps.append(im)

    trace = os.environ.get("MOE_TRACE") == "1"
    res = run_bass_kernel_spmd(nc, in_maps, core_ids=list(range(N_CORES)),
                               trace=trace)
    LAST.clear()
    LAST["exec_time_ns"] = res.exec_time_ns
    LAST["mean_exec_time_ns"] = res.mean_exec_time_ns
    LAST["results"] = res

    out = np.zeros((T, D), dtype=np.float32)
    for e in range(E):
        sel = expert_tok[e][:C]
        ye = res.results[e]["yT"][:, :len(sel)].T.astype(np.float32)
        pr = expert_prob[e][:len(sel)].copy()
        if fp8_tail and len(sel) > C - NT:
            # fp8 tail tile's yT carries the w2*512 pre-scale
            pr[C - NT:] /= SW1
        out[sel] += pr[:, None] * ye
        if len(expert_tok[e]) > C:  # host-side overflow (a few tokens)
            sel_o = expert_tok[e][C:]
            h = xt[sel_o] @ w1[e]
            h = h / (1.0 + np.exp(-h))
            yo = h @ w2[e]
            out[sel_o] += expert_prob[e][C:, None] * yo
    return out.reshape(B, S, D)
